# revision 3
# baseline (speedup 1.0000x reference)
"""Trainium2 Bass kernel v3 for nn_Attention_noZeromap (pooled-attention).

Contract: kernel(**inputs) -> full [8,128,128,128] f32, one sample per
NeuronCore (B=8 data-parallel), params folded on host.

v3 on top of v2's fused/guarded design:
  - Tap matmuls run in fp8e5 with DoubleRow pairs: taps (dh=-1,dw) and
    (dh=+1,dw) share one matmul (weights host-packed contiguously, the
    moving operand reads two disjoint y rows at i-stride 2S); the dh=0
    taps are plain fp8e5 matmuls.  27 bf16 tap-units become ~13.5.
  - All convs run in NORMAL [o,(h,w)] orientation; the v-path transposes
    to apply layout via per-tile PE transposes (bf16) fused into the
    consumer loops.  Stage-2's conv evacuates (w,h)-major so its
    transposes read contiguous tiles.
  - e5m2 underflow: tapk x16, tapvp x64, pqk x1024.  k/q/m paths are
    scale-invariant (l2n); the v-path descale is folded into the softmax
    output (a1/a2 carry 2^-6).
  - Stage-2 conv chunks depend only on neighboring y2 rows, so they
    pipeline into the apply/lnorm2 loop; the tail is just transposes +
    apply2 + residual + DMA.
"""

import numpy as np

import concourse.bass as bass
import concourse.mybir as mybir
from concourse import bass_utils
from concourse.tile import ScopedClock, TileContext

# --------------------------------------------------------------------------
# walrus in this environment rejects >1 sem-wait per instruction.


def _drain_and_barrier_split(self, tick_clock, wait_clock):
    drain_inst = self.nc.sync.drain()
    wait_clock.add_sem_waits(
        drain_inst.ins, ScopedClock({None: tick_clock.global_clock})
    )
    si = drain_inst.ins.sync_info
    if si is not None and si.on_wait and len(si.on_wait) > 1:
        waits = list(si.on_wait)
        si.on_wait = waits[:1]
        for w in waits[1:]:
            nop = self.nc.sync.nop(nofuse=True)
            nop.ins.sync_info = mybir.SyncInfo(on_wait=[w], on_update=[])
    self.nc.all_engine_barrier()
    assert self.sems is not None
    popped = self.nc._tile_sem_poison_stack.pop()
    assert popped is self._sem_poison
    self.nc.clear_and_free_semaphores(list(self.sems.allocated().values()))
    self.nc.all_engine_barrier()


TileContext._drain_and_barrier = _drain_and_barrier_split

_WAIT_LIMIT = 1


def _split_excess_waits(raw: bytes) -> bytes:
    import json

    m = json.loads(raw)
    ctr = 0
    for fn in m["functions"]:
        for blk in fn["blocks"]:
            out = []
            for inst in blk["instructions"]:
                si = inst.get("sync_info")
                ow = (si or {}).get("on_wait") or []
                if len(ow) > _WAIT_LIMIT:
                    keep, extra = ow[-_WAIT_LIMIT:], ow[: -_WAIT_LIMIT]
                    for w in extra:
                        ctr += 1
                        out.append({
                            "name": f"I-wsplit-{ctr}",
                            "opcode": "NoOp",
                            "engine": inst["engine"],
                            "ins": [], "outs": [],
                            "sync_info": {"on_update": [], "on_wait": [w]},
                            "debug": inst.get("debug", 0),
                        })
                    si["on_wait"] = keep
                out.append(inst)
            blk["instructions"] = out
    return json.dumps(m).encode()


import bass_rust

# --------------------------------------------------------------------------

P = 128
C = 128
H = 128
W = 128
HW = H * W
CH = 512
NCH = HW // CH
S = 130
D0 = S + 1
YLEN = 131 * S + 4
GU = 256
EPS_LN = 1e-5
BF = mybir.dt.bfloat16
F32 = mybir.dt.float32
FP8 = mybir.dt.float8e5
DRMODE = mybir.MatmulPerfMode.DoubleRow
AX = mybir.AxisListType
ALU = mybir.AluOpType
ACTF = mybir.ActivationFunctionType

KSC = 16.0     # tapk scale (l2n-invariant)
VSC = 64.0     # tapvp scale (descaled via a1/a2)
PSC = 1024.0   # pqk scale (l2n-invariant)


def _host_consts(inputs):
    f = lambda k: np.asarray(inputs[k], np.float32)
    ln_w, ln_b = f("ln_w"), f("ln_b")
    qkv_w = f("qkv_w")[:, :, 0, 0]
    qkv_b = f("qkv_b")
    dw_w = f("dw_w")[:, 0]
    dw_b = f("dw_b")
    proj_w = f("proj_w")[:, :, 0, 0]
    proj_b = f("proj_b")

    assert np.all(qkv_b == 0) and np.all(dw_b == 0) and np.all(ln_b == 0)
    assert np.all(proj_b == 0)

    Wg = qkv_w * ln_w[None, :]
    Wq, Wk, Wv = Wg[:C], Wg[C: 2 * C], Wg[2 * C:]
    W2q = Wq - Wq.mean(axis=1, keepdims=True)
    W2k = Wk - Wk.mean(axis=1, keepdims=True)
    W2v = Wv - Wv.mean(axis=1, keepdims=True)

    Kq = dw_w[:C].reshape(C, 9)
    Kk = dw_w[C: 2 * C].reshape(C, 9)
    Kv = dw_w[2 * C:].reshape(C, 9)

    def tap_k(t):
        return W2k.T * Kk[None, :, t].reshape(1, C)

    def tap_v(t):
        return (W2v.T * Kv[None, :, t].reshape(1, C)) @ proj_w.T

    # pairs: for dw block b (dw=b-1): [tap(-1,dw) | tap(+1,dw)] (contiguous)
    def pack(tapf, sc):
        pair = np.concatenate(
            [np.concatenate([tapf(0 * 3 + b), tapf(2 * 3 + b)], axis=1)
             for b in range(3)], axis=1) * sc        # [C, 768]
        sing = np.concatenate(
            [tapf(1 * 3 + b) for b in range(3)], axis=1) * sc  # [C, 384]
        return pair, sing

    tapk_pair, tapk_sing = pack(tap_k, KSC)
    tapvp_pair, tapvp_sing = pack(tap_v, VSC)

    PQK = np.zeros((C, 18), np.float32)
    for t in range(9):
        PQK[:, t] = (W2q.T @ Kq[:, t]) / C
        PQK[:, 9 + t] = (W2k.T @ Kk[:, t]) / C

    AQ = np.stack([Kq[:, dw] + Kq[:, 3 + dw] + Kq[:, 6 + dw] for dw in range(3)], 1)
    EQ0 = -Kq[:, 6:9]
    EQ127 = -Kq[:, 0:3]

    # unscaled full tap mats for the numpy goldens (not uploaded)
    g_tapk = np.concatenate([tap_k(t) for t in range(9)], axis=1)
    g_tapvp = np.concatenate([tap_v(t) for t in range(9)], axis=1)

    return {
        "tapk_pair": tapk_pair, "tapk_sing": tapk_sing,
        "tapvp_pair": tapvp_pair, "tapvp_sing": tapvp_sing,
        "wq_lhsT": W2q.T.copy(), "wq8": W2q.T.copy(),
        "ones": np.ones((C, P), np.float32),
        "pqk8": PQK * PSC,
        "ident": np.eye(P, dtype=np.float32),
        "aq": AQ, "eq0": EQ0, "eq127": EQ127,
        "epsln": np.full((P, 1), EPS_LN, np.float32),
        "eps24": np.full((P, 1), 1e-24, np.float32),
        "temp1": float(f("temp1").reshape(-1)[0]),
        "temp2": float(f("temp2").reshape(-1)[0]),
        "g_tapk": g_tapk, "g_tapvp": g_tapvp,
    }


CONST_SPECS = {
    "tapk_pair": ([C, 768], FP8), "tapk_sing": ([C, 384], FP8),
    "tapvp_pair": ([C, 768], FP8), "tapvp_sing": ([C, 384], FP8),
    "wq_lhsT": ([C, P], BF), "wq8": ([C, P], FP8),
    "ones": ([C, P], BF), "pqk8": ([C, 18], FP8), "ident": ([P, P], BF),
    "aq": ([C, 3], F32), "eq0": ([C, 3], F32), "eq127": ([C, 3], F32),
    "epsln": ([P, 1], F32), "eps24": ([P, 1], F32),
}


class Ctx:
    def __init__(self, nc, tc, cst, dbg):
        self.nc = nc
        self.tc = tc
        self.cst = cst
        self.dbg = dbg
        self.smalls = None

    def dump(self, name, ap):
        if name in self.dbg:
            self.nc.sync.dma_start(self.dbg[name][:], ap)


def g_rows(t, j, dh=0, dw=0, rows=4):
    base = D0 + (4 * j + dh) * S + dw
    return t[:, base: base + rows * S].rearrange(
        "p (h w) -> p h w", w=S)[:, :, 0:128]


def g_tile(t, h, dh=0, dw=0):
    q = D0 + (h + dh) * S + dw
    return t[:, q: q + 128]


def g_pair(t, j, r, dw):
    """[p, 2, 128] moving operand: rows (4j+r-1) and (4j+r+1), cols +dw."""
    base = D0 + (4 * j + r - 1) * S + dw
    v = t[:, base: base + 2 * S + 128].copy()
    v.ap = bass_rust.VecI64Pair([[YLEN, P], [2 * S, 2], [1, 128]])
    return v


def init_guards(ctx, t):
    nc = ctx.nc
    nc.gpsimd.memset(t[:, 0:D0], 0.0)
    tail = D0 + 127 * S + 128
    nc.gpsimd.memset(t[:, tail:YLEN], 0.0)
    gaps = t[:, D0 + 128: D0 + 128 + 127 * S].rearrange(
        "p (h w) -> p h w", w=S)[:, :, 0:2]
    nc.gpsimd.memset(gaps, 0.0)


def emit_l2n_rows(ctx, src_ap, out_bf, tag):
    nc = ctx.nc
    sm = ctx.smalls
    scr = sm.tile([P, P], F32, tag="l2scr", name=f"l2scr_{tag}")
    ss = sm.tile([P, 1], F32, tag="l2ss", name=f"l2ss_{tag}")
    nc.vector.tensor_mul(scr[:], src_ap, src_ap)
    nc.vector.tensor_reduce(ss[:], scr[:], AX.X, ALU.add)
    lnv = sm.tile([P, 1], F32, tag="l2ln", name=f"l2ln_{tag}")
    nc.scalar.activation(lnv[:], ss[:], ACTF.Ln, bias=ctx.cst["eps24"])
    inv = sm.tile([P, 1], F32, tag="l2i", name=f"l2i_{tag}")
    nc.scalar.activation(inv[:], lnv[:], ACTF.Exp, scale=-0.5)
    nc.vector.tensor_scalar(out_bf[:], src_ap, inv[:], None, ALU.mult)


def emit_softmax(ctx, logits_ps, temp, out_bf, tag, post_scale=1.0):
    nc = ctx.nc
    sm = ctx.smalls
    mx = sm.tile([P, 1], F32, tag="smx", name=f"smx_{tag}")
    nc.vector.tensor_reduce(mx[:], logits_ps[:], AX.X, ALU.max)
    nb = sm.tile([P, 1], F32, tag="snb", name=f"snb_{tag}")
    nc.vector.tensor_scalar(nb[:], mx[:], -temp, None, ALU.mult)
    e = sm.tile([P, P], F32, tag="sexp", name=f"sexp_{tag}")
    nc.scalar.activation(e[:], logits_ps[:], ACTF.Exp, bias=nb[:], scale=temp)
    s = sm.tile([P, 1], F32, tag="ssum", name=f"ssum_{tag}")
    nc.vector.tensor_reduce(s[:], e[:], AX.X, ALU.add)
    r = sm.tile([P, 1], F32, tag="srcp", name=f"srcp_{tag}")
    nc.vector.reciprocal(r[:], s[:])
    if post_scale != 1.0:
        nc.vector.tensor_scalar(r[:], r[:], post_scale, None, ALU.mult)
    nc.vector.tensor_scalar(out_bf[:], e[:], r[:], None, ALU.mult)


STOP_AT = None


def _stop(ctx, name, out_d):
    if STOP_AT == name:
        with ctx.tc.tile_pool(name="stopz", bufs=1) as zp:
            z = zp.tile([P, HW], BF, tag="z")
            ctx.nc.vector.memset(z[:], 0.0)
            ctx.nc.sync.dma_start(out_d[:], z[:])
        return True
    return False


def build_nc(consts, debug=()):
    nc = bass.Bass("TRN2")
    x_d = nc.dram_tensor("x", [P, HW], F32, kind="ExternalInput")
    out_d = nc.dram_tensor("out", [P, HW], BF, kind="ExternalOutput")
    cst_d = {
        n: nc.dram_tensor(n, shp, F32 if dt == FP8 else dt,
                          kind="ExternalInput")
        for n, (shp, dt) in CONST_SPECS.items()
    }
    dbg_d = {}
    for name, shp, dt in debug:
        dbg_d[name] = nc.dram_tensor(
            f"dbg_{name}", shp,
            {"bf": BF, "f8": FP8}.get(dt, F32),
            kind="ExternalOutput")

    with TileContext(nc, pool_alloc_mode="queue") as tc:
        with tc.tile_pool(name="consts", bufs=1) as cp, \
             tc.tile_pool(name="smalls", bufs=1) as smalls:
            cst = {}
            with tc.tile_pool(name="cstg", bufs=2) as cstg:
                for n, (shp, dt) in CONST_SPECS.items():
                    t = cp.tile(shp, dt, tag=n, name=n)
                    if dt == FP8:
                        stg = cstg.tile(shp, F32, tag="cstg", name=f"cs_{n}")
                        nc.sync.dma_start(stg[:], cst_d[n][:])
                        nc.scalar.copy(t[:], stg[:])
                    else:
                        nc.sync.dma_start(t[:], cst_d[n][:])
                    cst[n] = t[:]
            ctx = Ctx(nc, tc, cst, dbg_d)
            ctx.smalls = smalls
            _emit_model(ctx, x_d, out_d, consts)
    orig_to_json = nc.to_json_bytes
    nc.to_json_bytes = lambda: _split_excess_waits(orig_to_json())
    return nc


def _dump_g(ctx, name, gt):
    if name in ctx.dbg:
        v = gt[:, D0: D0 + 128 * S].rearrange(
            "p (h w) -> p h w", w=S)[:, :, 0:128]
        ctx.nc.sync.dma_start(
            ctx.dbg[name][:].rearrange("p (h w) -> p h w", w=128), v)


def emit_lnorm_chunk(ctx, j, xv, yg, ssp, rsp, tag):
    """One lnorm chunk: sq (DVE), colsum (PE), Ln+Exp (ACT), y=x*rstd (DVE,
    fp8e5 out into guarded yg)."""
    nc = ctx.nc
    sq = rsp.tile([P, CH], BF, tag="sq", name=f"sq{tag}{j}")
    sqv = sq[:].rearrange("p (h w) -> p h w", w=128)
    nc.vector.tensor_tensor(sqv, xv, xv, ALU.mult)
    ss = ssp.tile([P, CH], F32, tag="ss", name=f"ss{tag}{j}")
    nc.tensor.matmul(ss[:], ctx.cst["ones"], sq[:])
    lnv = rsp.tile([P, CH], F32, tag="lnv", name=f"lnv{tag}{j}")
    nc.scalar.activation(lnv[:], ss[:], ACTF.Ln,
                         bias=ctx.cst["epsln"], scale=1.0 / C)
    rst = rsp.tile([P, CH], BF, tag="sqr", name=f"sqr{tag}{j}")
    nc.scalar.activation(rst[:], lnv[:], ACTF.Exp, scale=-0.5)
    rv = rst[:].rearrange("p (h w) -> p h w", w=128)
    nc.vector.tensor_tensor(g_rows(yg, j), xv, rv, ALU.mult)


NO_DR = False  # plain-fp8 fallback for the DoubleRow pair matmuls (debug)


def emit_conv_chunk(ctx, j, yg, pool, pair_c, sing_c, nm):
    """Fused 3x3+1x1 conv chunk j in fp8e5: 3 plain dh=0 taps + 12 DoubleRow
    pair matmuls -> psum [P, CH] f32 (accumulated).  Returns the psum tile."""
    nc = ctx.nc
    ps = pool.tile([P, CH], F32, tag="cv", name=f"{nm}{j}")
    first = True
    for b in range(3):
        nc.tensor.matmul(ps[:], sing_c[:, b * C: (b + 1) * C],
                         g_rows(yg, j, 0, b - 1),
                         start=first, stop=False,
                         skip_group_check=True)
        first = False
    if NO_DR:
        for k, (dh, b) in enumerate([(dh, b) for dh in (-1, 1)
                                     for b in range(3)]):
            off = 0 if dh == -1 else 128
            nc.tensor.matmul(ps[:], pair_c[:, b * 256 + off:
                                           b * 256 + off + 128],
                             g_rows(yg, j, dh, b - 1),
                             start=False, stop=(k == 5),
                             skip_group_check=True)
        return ps
    for r in range(4):
        for b in range(3):
            last = (r == 3 and b == 2)
            lhsT = pair_c[:, b * 256: (b + 1) * 256].rearrange(
                "p (i m) -> p i m", i=2)
            nc.tensor.matmul(ps[:, r * P: (r + 1) * P], lhsT,
                             g_pair(yg, j, r, b - 1),
                             perf_mode=DRMODE,
                             start=False, stop=last, skip_group_check=True)
    return ps


def _emit_model(ctx, x_d, out_d, consts):
    nc, tc = ctx.nc, ctx.tc
    cst = ctx.cst
    sm = ctx.smalls

    with tc.tile_pool(name="big1", bufs=1) as big1, \
         tc.tile_pool(name="pvdp", bufs=1) as pvdp:
        out1 = big1.tile([P, YLEN], BF, tag="out1", name="out1")
        pvd = pvdp.tile([P, HW], BF, tag="pvd", name="pvd")
        init_guards(ctx, out1)

        # ================= stage 1: lnorm + kd + pvd ======================
        with tc.tile_pool(name="s1", bufs=1) as s1p:
            yg = s1p.tile([P, YLEN], FP8, tag="yg", name="yg")
            init_guards(ctx, yg)
            with tc.tile_pool(name="xf1", bufs=2) as xfp, \
                 tc.tile_pool(name="rs1", bufs=4) as rsp, \
                 tc.tile_pool(name="ss1", bufs=2, space="PSUM") as ssp, \
                 tc.tile_pool(name="kbufp", bufs=1) as kbufp, \
                 tc.tile_pool(name="cvps", bufs=3, space="PSUM") as cvps:
                kbuf = kbufp.tile([P, HW // 2], BF, tag="kbuf", name="kbuf")
                ybuf = kbufp.tile([P, HW // 2], BF, tag="ybuf", name="ybuf")
                xq = None
                for j in range(NCH + 2):
                    if j < NCH:
                        if j % 4 == 0:
                            xq = xfp.tile([P, 4 * CH], F32, tag="xq",
                                          name=f"xq{j}")
                            nc.sync.dma_start(
                                xq[:], x_d[:, j * CH: (j + 4) * CH])
                        xv = xq[:, (j % 4) * CH: (j % 4 + 1) * CH].rearrange(
                            "p (h w) -> p h w", w=128)
                        emit_lnorm_chunk(ctx, j, xv, yg, ssp, rsp, "s1")
                        if j >= 16:
                            jj = j - 16
                            ov = ybuf[:, jj * CH: (jj + 1) * CH].rearrange(
                                "p (h w) -> p h w", w=128)
                            nc.vector.tensor_tensor(
                                ov, g_rows(yg, jj), g_rows(yg, j), ALU.add)
                    if 1 <= j < NCH + 1:
                        jj = j - 1
                        ps = emit_conv_chunk(ctx, jj, yg, cvps,
                                             cst["tapk_pair"],
                                             cst["tapk_sing"], "kd")
                        sl = slice((jj % 16) * CH, (jj % 16 + 1) * CH)
                        if jj < 16:
                            nc.scalar.copy(kbuf[:, sl], ps[:])
                        else:
                            nc.vector.tensor_tensor(
                                kbuf[:, sl], ps[:], kbuf[:, sl], ALU.max)
                    if j >= 2:
                        jj = j - 2
                        ps = emit_conv_chunk(ctx, jj, yg, cvps,
                                             cst["tapvp_pair"],
                                             cst["tapvp_sing"], "pv")
                        if jj % 2 == 0:
                            nc.scalar.copy(pvd[:, jj * CH: (jj + 1) * CH],
                                           ps[:])
                        else:
                            nc.vector.tensor_copy(
                                pvd[:, jj * CH: (jj + 1) * CH], ps[:])
                _dump_g(ctx, "y1", yg)
                if _stop(ctx, "lnorm", out_d):
                    return

                # tree-max kbuf -> kmax -> k1
                n = HW // 4
                nc.vector.tensor_tensor(
                    kbuf[:, :n], kbuf[:, :n], kbuf[:, n: 2 * n], ALU.max)
                while n > 256:
                    h = n // 2
                    nc.vector.tensor_tensor(
                        kbuf[:, :h], kbuf[:, :h], kbuf[:, h: 2 * h], ALU.max)
                    n = h
                kmaxf = sm.tile([P, P], F32, tag="kmaxf")
                nc.vector.tensor_tensor(
                    kmaxf[:], kbuf[:, :128], kbuf[:, 128:256], ALU.max)
                k1 = sm.tile([P, P], BF, tag="k1")
                emit_l2n_rows(ctx, kmaxf[:], k1, "k1")
                ctx.dump("k1", kmaxf[:])

            # ---- pooled q + a1 (scoped small psum) ----
            with tc.tile_pool(name="ps_q", bufs=1, space="PSUM") as psq:
                n = HW // 4
                nc.vector.tensor_tensor(
                    ybuf[:, :n], ybuf[:, :n], ybuf[:, n: 2 * n], ALU.add)
                while n > 128:
                    h = n // 2
                    nc.vector.tensor_tensor(
                        ybuf[:, :h], ybuf[:, :h], ybuf[:, h: 2 * h], ALU.add)
                    n = h
                sq_ps = psq.tile([P, 384], F32, tag="pss", name="sq_ps")
                nc.tensor.matmul(sq_ps[:, 0:128], cst["wq_lhsT"],
                                 ybuf[:, 0:128])
                nc.tensor.matmul(sq_ps[:, 128:256], cst["wq8"], g_tile(yg, 0))
                nc.tensor.matmul(sq_ps[:, 256:384], cst["wq8"],
                                 g_tile(yg, 127))
                tg = sm.tile([P, 3 * 132], F32, tag="tg")
                nc.vector.memset(tg[:], 0.0)
                for dwi in range(3):
                    tsl = tg[:, dwi * 132 + 1: dwi * 132 + 129]
                    nc.vector.tensor_scalar(
                        tsl, sq_ps[:, 0:128], cst["aq"][:, dwi: dwi + 1],
                        None, ALU.mult)
                    nc.vector.scalar_tensor_tensor(
                        tsl, sq_ps[:, 128:256], cst["eq0"][:, dwi: dwi + 1],
                        tsl, ALU.mult, ALU.add)
                    nc.vector.scalar_tensor_tensor(
                        tsl, sq_ps[:, 256:384], cst["eq127"][:, dwi: dwi + 1],
                        tsl, ALU.mult, ALU.add)
                q1pre = sm.tile([P, P], F32, tag="q1pre")
                nc.vector.tensor_add(q1pre[:], tg[:, 0:128],
                                     tg[:, 132 + 1: 132 + 129])
                nc.vector.tensor_add(q1pre[:], q1pre[:],
                                     tg[:, 2 * 132 + 2: 2 * 132 + 130])
                ctx.dump("q1pre", q1pre[:])
                q1 = sm.tile([P, P], BF, tag="q1")
                emit_l2n_rows(ctx, q1pre[:], q1, "q1")

                lg_ps = psq.tile([P, P], F32, tag="pss", name="lg_ps")
                nc.tensor.matmul(lg_ps[:], q1[:], k1[:])
                a1 = sm.tile([P, P], BF, tag="a1")
                emit_softmax(ctx, lg_ps, consts["temp1"], a1, "a1",
                             post_scale=1.0 / VSC)
                ctx.dump("a1", a1[:])
            if _stop(ctx, "a1", out_d):
                return

        # ======== apply1 + lnorm2 + pqk + pvd2 (fused pipeline) ==========
        with tc.tile_pool(name="s2", bufs=1) as s2p, \
             tc.tile_pool(name="pvdtp", bufs=1) as pvdtp:
            yg2 = s2p.tile([P, YLEN], FP8, tag="yg2", name="yg2")
            init_guards(ctx, yg2)
            pvdt = pvdtp.tile([P, HW], BF, tag="pvdt", name="pvdt")
            pvd2 = pvdtp.tile([P, HW], BF, tag="pvd2", name="pvd2")
            pv2v = pvd2[:].rearrange("p (w h) -> p w h", h=128)
            with tc.tile_pool(name="mstg", bufs=2) as mstg, \
                 tc.tile_pool(name="mdram", bufs=1, space="DRAM") as mdram:
                mscr = mdram.tile([18, GU + HW + GU], BF, tag="mscr",
                                  name="mscr")
                mz = mstg.tile([18, GU], BF, tag="mz", name="mz")
                nc.vector.memset(mz[:], 0.0)
                nc.sync.dma_start(mscr[:, 0:GU], mz[:])
                nc.sync.dma_start(mscr[:, GU + HW:], mz[:])
                with tc.tile_pool(name="xf2", bufs=2) as xfp2, \
                     tc.tile_pool(name="rs2", bufs=4) as rsp2, \
                     tc.tile_pool(name="ss2", bufs=2, space="PSUM") as ssp2, \
                     tc.tile_pool(name="app", bufs=2, space="PSUM") as app, \
                     tc.tile_pool(name="tp1", bufs=1, space="PSUM") as tp1, \
                     tc.tile_pool(name="mps", bufs=1, space="PSUM") as mps, \
                     tc.tile_pool(name="cv2", bufs=2, space="PSUM") as cv2:
                    xq = None
                    for j in range(NCH + 3):
                        if j < NCH:
                            if j % 2 == 0:
                                xq = xfp2.tile([P, 2 * CH], F32, tag="xq",
                                               name=f"xq2{j}")
                                nc.sync.dma_start(
                                    xq[:], x_d[:, j * CH: (j + 2) * CH])
                            # transpose pvd tiles -> pvdt chunk j
                            tps = tp1.tile([P, CH], BF, tag="tp",
                                           name=f"tp{j}")
                            for hi in range(4):
                                h = 4 * j + hi
                                nc.tensor.transpose(
                                    tps[:, hi * P: (hi + 1) * P],
                                    pvd[:, h * P: (h + 1) * P], cst["ident"])
                            nc.vector.tensor_copy(
                                pvdt[:, j * CH: (j + 1) * CH], tps[:])
                            ps = app.tile([P, CH], F32, tag="ap",
                                          name=f"ap{j}")
                            for hi in range(4):
                                h = 4 * j + hi
                                nc.tensor.matmul(
                                    ps[:, hi * P: (hi + 1) * P],
                                    pvdt[:, h * P: (h + 1) * P], a1[:],
                                    skip_group_check=True)
                            psv = ps[:].rearrange("p (h w) -> p h w", w=128)
                            xv = xq[:, (j % 2) * CH: (j % 2 + 1) * CH]\
                                .rearrange("p (h w) -> p h w", w=128)
                            nc.vector.tensor_tensor(
                                g_rows(out1, j), psv, xv, ALU.add)
                        if 1 <= j < NCH + 1:
                            jj = j - 1
                            emit_lnorm_chunk(ctx, jj, g_rows(out1, jj), yg2,
                                             ssp2, rsp2, "s2")
                        if 2 <= j < NCH + 2:
                            jj = j - 2
                            psm = mps.tile([18, CH], F32, tag="mps",
                                           name=f"mps{jj}")
                            nc.tensor.matmul(psm[:], cst["pqk8"],
                                             g_rows(yg2, jj))
                            mst = mstg.tile([18, CH], BF, tag="mst",
                                            name=f"mst{jj}")
                            nc.scalar.copy(mst[:], psm[:])
                            nc.sync.dma_start(
                                mscr[:, GU + jj * CH: GU + (jj + 1) * CH],
                                mst[:])
                        if 3 <= j < NCH + 3:
                            jj = j - 3
                            ps = emit_conv_chunk(ctx, jj, yg2, cv2,
                                                 cst["tapvp_pair"],
                                                 cst["tapvp_sing"], "pv2")
                            # strided evac -> pvd2 (w,h)-major
                            psv = ps[:].rearrange("p (h w) -> p h w", w=128)
                            ov = pv2v[:, :, 4 * jj: 4 * jj + 4].transpose(
                                [0, 2, 1])
                            nc.scalar.copy(ov, psv)
                    _dump_g(ctx, "out1", out1)
                    _dump_g(ctx, "y2", yg2)
                    if _stop(ctx, "out1", out_d):
                        return

                # ---- q2/k2 repartition reads ----
                qt = sm.tile([P, 9 * P], BF, tag="qtiles")
                kt = sm.tile([P, 9 * P], BF, tag="ktiles")
                for t in range(9):
                    dh, dw = t // 3 - 1, t % 3 - 1
                    off = GU + dh * 128 + dw
                    nc.sync.dma_start(
                        qt[:, t * P: (t + 1) * P],
                        mscr[t, off: off + HW].rearrange("(h w) -> h w", h=P))
                    nc.sync.dma_start(
                        kt[:, t * P: (t + 1) * P],
                        mscr[9 + t, off: off + HW].rearrange(
                            "(h w) -> h w", h=P))
            with tc.tile_pool(name="ps_a2", bufs=1, space="PSUM") as psa2:
                q2t = k2t = None
                for nm in ("q2", "k2"):
                    tt = qt if nm == "q2" else kt
                    acc = sm.tile([P, P], F32, tag=f"{nm}pre", name=f"{nm}pre")
                    nc.vector.tensor_add(acc[:], tt[:, 0:P], tt[:, P: 2 * P])
                    for t in range(2, 9):
                        nc.vector.tensor_add(acc[:], acc[:],
                                             tt[:, t * P: (t + 1) * P])
                    for w, bad_dw in ((0, 0), (127, 2)):
                        first = True
                        for t in range(9):
                            if t % 3 == bad_dw:
                                continue
                            src = tt[:, t * P + w: t * P + w + 1]
                            if first:
                                nc.vector.tensor_copy(acc[:, w: w + 1], src)
                                first = False
                            else:
                                nc.vector.tensor_add(
                                    acc[:, w: w + 1], acc[:, w: w + 1], src)
                    ctx.dump(f"{nm}pre", acc[:])
                    nbf = sm.tile([P, P], BF, tag=nm, name=nm)
                    emit_l2n_rows(ctx, acc[:], nbf, nm)
                    pst = psa2.tile([P, P], BF, tag="pss", name=f"{nm}tp")
                    nc.tensor.transpose(pst[:], nbf[:], cst["ident"])
                    ntp = sm.tile([P, P], BF, tag=f"{nm}T", name=f"{nm}T")
                    nc.vector.tensor_copy(ntp[:], pst[:])
                    if nm == "q2":
                        q2t = ntp
                    else:
                        k2t = ntp

                lg_ps = psa2.tile([P, P], F32, tag="pss", name="lg_ps2")
                nc.tensor.matmul(lg_ps[:], q2t[:], k2t[:])
                a2 = sm.tile([P, P], BF, tag="a2")
                emit_softmax(ctx, lg_ps, consts["temp2"], a2, "a2",
                             post_scale=1.0 / VSC)
                ctx.dump("a2", a2[:])
            if _stop(ctx, "a2", out_d):
                return

            # ======== transposes2 + apply2 + residual -> out ==============
            with tc.tile_pool(name="tp2", bufs=2, space="PSUM") as tp2, \
                 tc.tile_pool(name="ap2", bufs=2, space="PSUM") as ap2, \
                 tc.tile_pool(name="outp", bufs=4) as outp:
                pvd2t = pvdt  # reuse (apply1 is done with it)
                o1all = out1[:, D0: D0 + 128 * S].rearrange(
                    "p (g w) -> p g w", w=S)
                for j in range(NCH + 1):
                    if j < NCH:
                        tps = tp2.tile([P, CH], BF, tag="tp2", name=f"t2{j}")
                        for wi in range(4):
                            w = 4 * j + wi
                            nc.tensor.transpose(
                                tps[:, wi * P: (wi + 1) * P],
                                pvd2[:, w * P: (w + 1) * P], cst["ident"])
                        nc.scalar.copy(
                            pvd2t[:, j * CH: (j + 1) * CH], tps[:])
                    if j >= 1:
                        jj = j - 1
                        ps = ap2.tile([P, CH], F32, tag="a2p", name=f"a2p{jj}")
                        for wi in range(4):
                            w = 4 * jj + wi
                            nc.tensor.matmul(
                                ps[:, wi * P: (wi + 1) * P],
                                pvd2t[:, w * P: (w + 1) * P], a2[:],
                                skip_group_check=True)
                        st = outp.tile([P, CH], BF, tag="st", name=f"st{jj}")
                        psv = ps[:].rearrange("p (w g) -> p w g", g=128)
                        stv = st[:].rearrange("p (w g) -> p w g", g=128)
                        o1v = o1all[:, :, 4 * jj: 4 * jj + 4].transpose(
                            [0, 2, 1])
                        nc.vector.tensor_tensor(stv, psv, o1v, ALU.add)
                        nc.sync.dma_start(
                            out_d[:, jj * CH: (jj + 1) * CH], st[:])


# --------------------------------------------------------------------------


def kernel(**inputs):
    B = 8
    trace = bool(inputs.pop("_trace", False))
    x = np.asarray(inputs["x"], np.float32)
    consts = _host_consts(inputs)
    nc = build_nc(consts)

    import ml_dtypes
    const_arrays = {}
    for n, (shp, dt) in CONST_SPECS.items():
        a = np.asarray(consts[n], np.float32).reshape(shp)
        if dt == BF:
            a = a.astype(ml_dtypes.bfloat16)
        const_arrays[n] = a

    in_maps = []
    for b in range(B):
        mm = {"x": x[b].reshape(P, HW).copy()}
        mm.update(const_arrays)
        in_maps.append(mm)

    res = bass_utils.run_bass_kernel_spmd(nc, in_maps, core_ids=list(range(B)),
                                          trace=trace)
    return np.stack([
        np.asarray(res.results[b]["out"], np.float32)
        .reshape(C, W, H).transpose(0, 2, 1)
        for b in range(B)
    ])


def check_build():
    rng = np.random.default_rng(0)
    fake = {
        "x": rng.normal(size=(8, C, H, W)).astype(np.float32),
        "ln_w": np.ones(C, np.float32), "ln_b": np.zeros(C, np.float32),
        "qkv_w": rng.normal(size=(3 * C, C, 1, 1)).astype(np.float32) * 0.02,
        "qkv_b": np.zeros(3 * C, np.float32),
        "dw_w": rng.normal(size=(3 * C, 1, 3, 3)).astype(np.float32) * 0.02,
        "dw_b": np.zeros(3 * C, np.float32),
        "proj_w": rng.normal(size=(C, C, 1, 1)).astype(np.float32) * 0.02,
        "proj_b": np.zeros(C, np.float32),
        "temp1": np.ones((1, 1), np.float32),
        "temp2": np.ones((1, 1), np.float32),
    }
    build_nc(_host_consts(fake))
    print("build OK")


if __name__ == "__main__":
    check_build()
